# revision 1
# baseline (speedup 1.0000x reference)
"""Trainium2 Bass kernel for nn_Block_73615739454081 (tri-block sparse attention + FFN).

Contract: kernel(**inputs) takes FULL unsharded inputs (as produced by
setup_inputs()) and returns the FULL [1, N, D] float32 output.

Strategy (8 NeuronCores, SPMD):
  - Shard the block axis: 161 blocks of 256 tokens. Each core runs an identical
    program over 21 "local" blocks plus a 1-block halo on each side (23-block
    slab). Adjacent cores overlap by one block; the gather keeps a disjoint
    20/20/.../21 split. No collectives: the halo is materialized host-side.
  - Conditioning (scale/offset from global_norm_conditioning) and the attention
    1/sqrt(d) scale are folded into the weight matrices on the host, so the
    device only computes plain LayerNorm stats.
  - Attention is computed in transposed (feature-major) layouts throughout:
    qT/kT = [HD, tokens], scores ST = [keys, q], so no transposes are needed
    inside the attention core. Softmax skips max-subtraction (logits are O(1));
    denominators come from ones-matmuls, broadcast back via a K=1 matmul.
  - Key-validity masking (padding tokens / missing neighbor blocks) is data
    driven: exp(scores) rows are multiplied by a per-core 0/1 key-mask at the
    (statically known) slab edges.
  - All matmuls run in bf16 (fp32 "HIGH" mode streams at ~half rate and
    disables fast weight load); accumulation stays fp32 in PSUM, and the
    residual path stays fp32 in SBUF.
  - Each chain's softmax-normalize + projection + FFN is emitted after the
    NEXT block's QKV matmuls so the [1,qw] reciprocal latency hides behind
    independent tensor-engine work (keeps the PE HAM clock warm).
"""

import numpy as np

# ---------------------------------------------------------------- constants
N = 40962
D = 512
H = 4
HD = 128
BS = 256
NB = 161
NP = NB * BS
C = 16
FFW = 4 * D
SCALE = HD ** -0.5
EPS = 1e-5

NCORES = 8
LOCAL = 21                 # local blocks per core (uniform SPMD program)
SLAB = LOCAL + 2           # +1 halo block each side
SNODES = SLAB * BS         # 5888 slab tokens
KT = SNODES // 128         # 46 key tiles of 128 in the slab
OUT_NODES = LOCAL * BS     # 5376
STARTS = [0, 20, 40, 60, 80, 100, 120, 140]   # first local block per core
# kt tiles that can contain invalid keys on some core (slab edges):
FIXUP_KTS = (0, 1, KT - 4, KT - 3, KT - 2, KT - 1)

_PROG_CACHE = {}


# ---------------------------------------------------------------- device code
def _build_program(has_bf, has_bd, debug_stage=None, repeat=1):
    import concourse.bass as bass  # noqa: F401
    import concourse.mybir as mybir
    import concourse.tile as tile
    from concourse import bacc

    F32 = mybir.dt.float32
    BF16 = mybir.dt.bfloat16
    AF = mybir.ActivationFunctionType
    OP = mybir.AluOpType

    nc = bacc.Bacc("TRN2", target_bir_lowering=False, debug=False)

    def din(name, shape, dt):
        return nc.dram_tensor(name, shape, dt, kind="ExternalInput").ap()

    x_d = din("x_slab", [SNODES, D], F32)
    km_d = din("kmask", [128, KT], F32)
    wqT_d = din("wqT", [128, 4 * H * HD], BF16)
    wkT_d = din("wkT", [128, 4 * H * HD], BF16)
    wvN_d = din("wvN", [128, 4 * D], BF16)
    wfN_d = din("wfN", [128, H * D], BF16)
    wupT_d = din("wupT", [128, 4 * FFW], BF16)
    wdnN_d = din("wdnN", [128, 16 * D], BF16)
    cqB_d = din("cqB", [128, H], F32)
    ckB_d = din("ckB", [128, H], F32)
    cvB_d = din("cvB", [128, D], F32)
    cuB_d = din("cuB", [128, 16], F32)
    ident_d = din("ident", [128, 128], BF16)
    epsB_d = din("epsB", [128, 1], F32)
    if has_bf:
        bfB_d = din("bfB", [128, D], F32)
    if has_bd:
        bdB_d = din("bdB", [128, D], F32)
    out_d = nc.dram_tensor("out", [OUT_NODES, D], F32, kind="ExternalOutput").ap()

    with nc.allow_low_precision(reason="bf16 matmul operands by design"), \
         tile.TileContext(nc) as tc:
        with (
            tc.tile_pool(name="wconst", bufs=1) as wpool,
            tc.tile_pool(name="sb", bufs=2) as sb,
            tc.tile_pool(name="ps", bufs=2, space="PSUM") as ps,
        ):
            # ---------------- resident weights / constants
            # small constants first, then weights in first-use order so the
            # first phase1 blocks only on a small DMA prefix.
            epsB = wpool.tile([128, 1], F32, name="epsB")
            nc.scalar.dma_start(out=epsB, in_=epsB_d)
            ident = wpool.tile([128, 128], BF16, name="ident")
            nc.scalar.dma_start(out=ident, in_=ident_d)
            cqB = wpool.tile([128, H], F32, name="cqB")
            nc.scalar.dma_start(out=cqB, in_=cqB_d)
            ckB = wpool.tile([128, H], F32, name="ckB")
            nc.scalar.dma_start(out=ckB, in_=ckB_d)
            cvB = wpool.tile([128, D], F32, name="cvB")
            nc.scalar.dma_start(out=cvB, in_=cvB_d)
            cuB = wpool.tile([128, 16], F32, name="cuB")
            nc.scalar.dma_start(out=cuB, in_=cuB_d)
            kmt = wpool.tile([128, KT], F32, name="kmt")
            nc.scalar.dma_start(out=kmt, in_=km_d)
            if has_bf:
                bfB = wpool.tile([128, D], F32, name="bfB")
                nc.scalar.dma_start(out=bfB, in_=bfB_d)
            if has_bd:
                bdB = wpool.tile([128, D], F32, name="bdB")
                nc.scalar.dma_start(out=bdB, in_=bdB_d)
            wkT = wpool.tile([128, 4 * H * HD], BF16, name="wkT")
            nc.scalar.dma_start(out=wkT, in_=wkT_d)
            wvN = wpool.tile([128, 4 * D], BF16, name="wvN")
            nc.scalar.dma_start(out=wvN, in_=wvN_d)
            wqT = wpool.tile([128, 4 * H * HD], BF16, name="wqT")
            nc.scalar.dma_start(out=wqT, in_=wqT_d)
            wfN = wpool.tile([128, H * D], BF16, name="wfN")
            nc.scalar.dma_start(out=wfN, in_=wfN_d)
            wupT = wpool.tile([128, 4 * FFW], BF16, name="wupT")
            nc.scalar.dma_start(out=wupT, in_=wupT_d)
            wdnN = wpool.tile([128, 16 * D], BF16, name="wdnN")
            nc.scalar.dma_start(out=wdnN, in_=wdnN_d)

            kT_ring = {}
            v_ring = {}
            qT_ring = {}

            def layer_norm(src, nm, apply_on_act=False):
                """LN stats + apply: returns hn tile [128, 512] bf16."""
                stats = sb.tile([128, 6], F32, tag="stats", bufs=4, name=f"st{nm}")
                nc.vector.bn_stats(out=stats, in_=src)
                mv = sb.tile([128, 2], F32, tag="mv", bufs=4, name=f"mv{nm}")
                nc.vector.bn_aggr(out=mv, in_=stats)
                std = sb.tile([128, 1], F32, tag="std", bufs=4, name=f"sd{nm}")
                nc.scalar.activation(std, mv[:, 1:2], AF.Sqrt, bias=epsB)
                rstd = sb.tile([128, 1], F32, tag="rstd", bufs=4, name=f"rs{nm}")
                nc.vector.reciprocal(rstd, std)
                hn = sb.tile([128, D], BF16, tag="hn", bufs=8, name=f"hn{nm}")
                if apply_on_act:
                    # out = rstd*x - mu*rstd on ACT (Identity is in every
                    # table set) -> shortens the DVE serial chain in LN2.
                    nmr = sb.tile([128, 1], F32, tag="nmr", bufs=4,
                                  name=f"nm{nm}")
                    nc.vector.tensor_scalar(
                        nmr, mv[:, 0:1], rstd, -1.0, OP.mult, OP.mult
                    )
                    nc.scalar.activation(hn, src, AF.Identity, bias=nmr,
                                         scale=rstd)
                else:
                    nc.vector.tensor_scalar(
                        hn, src, mv[:, 0:1], rstd, OP.subtract, OP.mult
                    )
                return hn

            def phase1(j):
                """LN1 + transpose + q/k/v projections for slab block j."""
                r0 = j * BS
                hn = []
                for nt in range(2):
                    xt = sb.tile([128, D], F32, tag="xin", bufs=4, name=f"x{j}_{nt}")
                    nc.sync.dma_start(out=xt, in_=x_d[r0 + nt * 128:r0 + nt * 128 + 128, :])
                    hn.append(layer_norm(xt, f"1_{j}_{nt}"))
                # transpose hn -> h1T [128, 4*256] (d-tile-major, token minor)
                h1T = sb.tile([128, 4 * BS], BF16, tag="h1T", bufs=2, name=f"h1T{j}")
                for g in range(2):  # two psum tiles, each covers 2 d-tiles
                    pT = ps.tile([128, 512], BF16, tag="p1", bufs=2, name=f"pT{j}_{g}")
                    for dl in range(2):
                        dt = g * 2 + dl
                        for nt in range(2):
                            nc.tensor.transpose(
                                pT[:, dl * 256 + nt * 128:dl * 256 + nt * 128 + 128],
                                hn[nt][:, dt * 128:dt * 128 + 128],
                                ident,
                            )
                    if g == 0:
                        nc.vector.tensor_copy(h1T[:, 0:512], pT)
                    else:
                        nc.scalar.copy(h1T[:, 512:1024], pT)

                def proj_T(wT, cB, nm):
                    """qT/kT-style projection -> [128, H*256] bf16 tile."""
                    outt = sb.tile([128, H * BS], BF16, tag=f"{nm}ring",
                                   bufs=(3 if nm == "q" else 4), name=f"{nm}T{j}")
                    for g in range(2):
                        pQ = ps.tile([128, 512], F32, tag="p1", bufs=2,
                                     name=f"p{nm}{j}_{g}")
                        for hl in range(2):
                            h = g * 2 + hl
                            for kt in range(4):
                                nc.tensor.matmul(
                                    pQ[:, hl * 256:hl * 256 + 256],
                                    lhsT=wT[:, (kt * H + h) * 128:(kt * H + h) * 128 + 128],
                                    rhs=h1T[:, kt * 256:kt * 256 + 256],
                                    start=(kt == 0), stop=(kt == 3),
                                )
                        # bias-add + copy to sbuf, split DVE/ACT
                        for hl in range(2):
                            h = g * 2 + hl
                            dst = outt[:, h * 256:h * 256 + 256]
                            src = pQ[:, hl * 256:hl * 256 + 256]
                            if hl == 0:
                                nc.vector.tensor_scalar(
                                    dst, src, cB[:, h:h + 1], None, OP.add
                                )
                            else:
                                nc.scalar.add(dst, src, cB[:, h:h + 1])
                    return outt

                if 1 <= j <= LOCAL:
                    qT_ring[j] = proj_T(wqT, cqB, "q")
                kT_ring[j] = proj_T(wkT, ckB, "k")
                for nt in range(2):
                    pV = ps.tile([128, 512], F32, tag="p1", bufs=2, name=f"pV{j}_{nt}")
                    for kt in range(4):
                        nc.tensor.matmul(
                            pV,
                            lhsT=h1T[:, kt * 256 + nt * 128:kt * 256 + nt * 128 + 128],
                            rhs=wvN[:, kt * D:kt * D + D],
                            start=(kt == 0), stop=(kt == 3),
                        )
                    # v_aug layout: per-head 129-wide slot [ones | v_h] so the
                    # AV matmul's rhs yields the softmax denominator in col 0.
                    vt = sb.tile([128, H * (HD + 1)], BF16, tag="vring", bufs=8,
                                 name=f"v{j}_{nt}")
                    nc.gpsimd.memset(vt, 1.0)
                    for h in range(H):
                        nc.vector.tensor_tensor(
                            vt[:, h * 129 + 1:h * 129 + 129],
                            pV[:, h * 128:h * 128 + 128],
                            cvB[:, h * 128:h * 128 + 128], OP.add)
                    v_ring[(j, nt)] = vt

            def chain_attn(b0, nb):
                """Attention for local blocks b0..b0+nb-1.

                Scores stay [keys, q]; the AV matmul uses exp(S) as the
                STATIONARY operand against the augmented V ([ones | v_h]
                slots), producing o = [q, 1 + hd] per query subtile with the
                softmax denominator in column 0.  Normalization is then a
                [128,1] reciprocal + per-partition scale (DVE-only, so the
                tensor engine never waits on it), and a PE transpose brings
                o back to [hd, q] for the final projection.  Returns
                (oTn, finisher); the caller emits finisher() later so the
                last head's transposes hide behind independent matmuls.
                """
                qw = nb * BS
                nqs = 2 * nb
                kts = list(range(2 * b0, 2 * b0 + 2 * (nb + 2)))

                def _vbis(kt):
                    return [bi for bi in range(nb)
                            if 2 * (b0 + bi) <= kt <= 2 * (b0 + bi) + 5]

                # prefetch the residual rows rest_A will need, so the adds
                # there never wait on DMA latency
                xres_t = []
                for qs in range(nqs):
                    xres = sb.tile([128, D], F32, tag="xres", bufs=8,
                                   name=f"xr{b0}_{qs}")
                    r0 = (b0 + 1) * BS + qs * 128
                    nc.sync.dma_start(out=xres, in_=x_d[r0:r0 + 128, :])
                    xres_t.append(xres)

                oTn = []
                pend = None
                for h in range(H):
                    # ---- scores + exp for every kt of this head
                    Et = {}
                    for kt in kts:
                        vbis = _vbis(kt)
                        c0 = min(vbis) * BS
                        c1 = (max(vbis) + 1) * BS
                        STp = ps.tile([128, qw], F32, tag="st", bufs=2,
                                      name=f"S{b0}_{h}_{kt}")
                        ksrc = kT_ring[kt // 2][:, h * 256 + (kt % 2) * 128:
                                                h * 256 + (kt % 2) * 128 + 128]
                        for bi in vbis:
                            nc.tensor.matmul(
                                STp[:, bi * BS:bi * BS + BS],
                                lhsT=ksrc,
                                rhs=qT_ring[b0 + bi + 1][:, h * 256:h * 256 + 256],
                                start=True, stop=True,
                            )
                        E = sb.tile([128, qw], BF16, tag="E", bufs=10,
                                    name=f"E{b0}_{h}_{kt}")
                        nc.scalar.activation(E[:, c0:c1], STp[:, c0:c1], AF.Exp)
                        if kt in FIXUP_KTS:
                            nc.vector.tensor_scalar(
                                E[:, c0:c1], E[:, c0:c1],
                                kmt[:, kt:kt + 1], None, OP.mult
                            )
                        Et[kt] = E
                    if pend is not None:
                        pend()      # prev head's transposes, covered by scores
                        pend = None
                    # ---- AV': per query subtile, accumulate over its 6 kts
                    oah = [ps.tile([128, 2 * (HD + 1)], F32, tag="oacc", bufs=2,
                                   name=f"oa{b0}_{h}_{hf}") for hf in range(nb)]
                    for qs in range(nqs):
                        bi = qs // 2
                        myk = list(range(2 * (b0 + bi), 2 * (b0 + bi) + 6))
                        dst = oah[qs // 2][:, (qs % 2) * 129:(qs % 2) * 129 + 129]
                        for i, kt in enumerate(myk):
                            nc.tensor.matmul(
                                dst,
                                lhsT=Et[kt][:, qs * 128:qs * 128 + 128],
                                rhs=v_ring[(kt // 2, kt % 2)][:, h * 129:h * 129 + 129],
                                start=(i == 0), stop=(i == 5),
                            )
                    # ---- normalize on DVE (off the PE critical path)
                    oos = []
                    for qs in range(nqs):
                        src = oah[qs // 2][:, (qs % 2) * 129:(qs % 2) * 129 + 129]
                        rdn = sb.tile([128, 1], F32, tag="rdn", bufs=8,
                                      name=f"rd{b0}_{h}_{qs}")
                        nc.vector.reciprocal(rdn, src[:, 0:1])
                        oo = sb.tile([128, HD], BF16, tag="oon", bufs=8,
                                     name=f"oo{b0}_{h}_{qs}")
                        nc.vector.tensor_scalar(oo, src[:, 1:129], rdn, None,
                                                OP.mult)
                        oos.append(oo)
                    oT = sb.tile([128, qw], BF16, tag="oTn", bufs=8,
                                 name=f"oT{b0}_{h}")

                    def mk(h=h, oos=oos, oT=oT, nqs=nqs, qw=qw, b0=b0):
                        trp = ps.tile([128, qw], BF16, tag="p1", bufs=2,
                                      name=f"tr{b0}_{h}")
                        for qs in range(nqs):
                            nc.tensor.transpose(
                                trp[:, qs * 128:qs * 128 + 128], oos[qs], ident)
                        if h % 2 == 0:
                            nc.vector.tensor_copy(oT, trp)
                        else:
                            nc.scalar.copy(oT, trp)
                    pend = mk
                    oTn.append(oT)
                return (oTn, xres_t), pend

            def rest_A(b0, nb, oTn, xres_t):
                """Final projection + residual + LN2 for local blocks.

                The residual is folded into the PSUM accumulation via an
                identity matmul, so LN2 stats/apply read the PSUM tile
                directly and the DVE serial chain per subtile is minimal.
                """
                qw = nb * BS
                nqs = qw // 128
                r1 = []
                hn2 = []
                for qs in range(nqs):
                    at = ps.tile([128, D], F32, tag="ac", bufs=2, name=f"at{b0}_{qs}")
                    for h in range(H):
                        nc.tensor.matmul(
                            at,
                            lhsT=oTn[h][:, qs * 128:qs * 128 + 128],
                            rhs=wfN[:, h * D:h * D + D],
                            start=(h == 0), stop=(h == 3),
                        )
                    rt = sb.tile([128, D], F32, tag="r1", bufs=8, name=f"r1{b0}_{qs}")
                    nc.vector.tensor_tensor(rt, at, xres_t[qs], OP.add)
                    if has_bf:
                        nc.vector.tensor_tensor(rt, rt, bfB, OP.add)
                    r1.append(rt)
                    if debug_stage == "r1":
                        ro = b0 * BS + qs * 128
                        nc.sync.dma_start(out=out_d[ro:ro + 128, :], in_=rt)
                    else:
                        hn2.append(layer_norm(rt, f"2_{b0}_{qs}",
                                              apply_on_act=True))
                return (b0, nb, r1, hn2)

            def rest_B(state):
                """h2T transposes + FFN; emitted after the NEXT chain's
                attention so the LN2 serial chain hides behind matmuls."""
                b0, nb, r1, hn2 = state
                if debug_stage == "r1":
                    return
                qw = nb * BS
                nqs = qw // 128
                # HAM keepalive: a tiny matmul gated on the FIRST LN2 apply
                # fires mid-way through the LN2 PE-idle stretch, splitting it
                # below the ~3.4us window after which the PE clock would
                # re-throttle to half rate.
                ka = ps.tile([128, 128], F32, tag="ac", bufs=2, name=f"ka{b0}")
                nc.tensor.matmul(ka, lhsT=ident, rhs=hn2[0][:, 0:128],
                                 start=True, stop=True)
                h2T = []
                for dt in range(4):
                    hps = ps.tile([128, qw], BF16, tag="ac", bufs=2,
                                  name=f"hp{b0}_{dt}")
                    for qs in range(nqs):
                        nc.tensor.transpose(
                            hps[:, qs * 128:qs * 128 + 128],
                            hn2[qs][:, dt * 128:dt * 128 + 128],
                            ident,
                        )
                    ht = sb.tile([128, qw], BF16, tag="h2T", bufs=5, name=f"h2{b0}_{dt}")
                    if dt % 2 == 0:
                        nc.vector.tensor_copy(ht, hps)
                    else:
                        nc.scalar.copy(ht, hps)
                    h2T.append(ht)

                # FFN up + gelu (retain gl tiles), then down per q-subtile
                gl = []
                for fb in range(16):
                    g = ps.tile([128, qw], F32, tag="ac", bufs=2, name=f"g{b0}_{fb}")
                    for kt in range(4):
                        nc.tensor.matmul(
                            g,
                            lhsT=wupT[:, (kt * 16 + fb) * 128:(kt * 16 + fb) * 128 + 128],
                            rhs=h2T[kt],
                            start=(kt == 0), stop=(kt == 3),
                        )
                    gt = sb.tile([128, qw], BF16, tag="gl", bufs=17,
                                 name=f"gl{b0}_{fb}")
                    nc.scalar.activation(gt, g, AF.Gelu_apprx_tanh,
                                         bias=cuB[:, fb:fb + 1])
                    gl.append(gt)
                for qs in range(nqs):
                    y = ps.tile([128, D], F32, tag="ac", bufs=2, name=f"y{b0}_{qs}")
                    for fb in range(16):
                        nc.tensor.matmul(
                            y,
                            lhsT=gl[fb][:, qs * 128:qs * 128 + 128],
                            rhs=wdnN[:, fb * D:fb * D + D],
                            start=(fb == 0), stop=(fb == 15),
                        )
                    ot = sb.tile([128, D], F32, tag="outt", bufs=3,
                                 name=f"ot{b0}_{qs}")
                    nc.vector.tensor_tensor(ot, y, r1[qs], OP.add)
                    if has_bd:
                        nc.vector.tensor_tensor(ot, ot, bdB, OP.add)
                    ro = b0 * BS + qs * 128
                    nc.sync.dma_start(out=out_d[ro:ro + 128, :], in_=ot)

            # ---------------- emission
            # Pipeline: attention(c) -> [next phase1] -> finisher(c)+rest_A(c)
            # -> [attention(c+1)] -> rest_B(c).  Each DVE/ACT serial chain
            # (softmax normalize, LN2) is emitted behind a large block of
            # independent matmuls so the in-order tensor queue never stalls.
            for _rep in range(repeat):
                kT_ring.clear(); v_ring.clear(); qT_ring.clear()
                pending = None      # (finisher, (b0, nb, oTn)) awaiting rest
                for j in range(SLAB):
                    phase1(j)
                    if pending is not None and j < SLAB - 1:
                        fin, args = pending
                        fin()
                        rest_B(rest_A(*args))
                        pending = None
                    if j >= 3 and (j % 2) == 1 and (j - 3) // 2 <= 9:
                        b0 = 2 * ((j - 3) // 2)
                        (oTn, xres_t), fin = chain_attn(b0, 2)
                        pending = (fin, (b0, 2, oTn, xres_t))
                (last_oTn, last_xres), last_fin = chain_attn(20, 1)
                fin, args = pending
                fin()
                rest_B(rest_A(*args))
                last_fin()
                rest_B(rest_A(20, 1, last_oTn, last_xres))

    nc.compile()
    return nc


# ---------------------------------------------------------------- host side
def _prep(inputs):
    import ml_dtypes
    f8 = np.float64
    BF = ml_dtypes.bfloat16
    x = np.asarray(inputs["x"], np.float32).reshape(N, D)
    gnc = np.asarray(inputs["global_norm_conditioning"], np.float32)
    mask = np.asarray(inputs["mask"])
    wq = np.asarray(inputs["wq"], np.float32)
    wk = np.asarray(inputs["wk"], np.float32)
    wv = np.asarray(inputs["wv"], np.float32)
    w_final = np.asarray(inputs["w_final"], np.float32)
    b_final = np.asarray(inputs["b_final"], np.float32)
    w_up = np.asarray(inputs["w_up"], np.float32)
    b_up = np.asarray(inputs["b_up"], np.float32)
    w_down = np.asarray(inputs["w_down"], np.float32)
    b_down = np.asarray(inputs["b_down"], np.float32)
    w_cond = np.asarray(inputs["w_cond"], np.float32)
    b_cond = np.asarray(inputs["b_cond"], np.float32)

    so = gnc.astype(f8) @ w_cond.astype(f8) + b_cond.astype(f8)
    sc = 1.0 + so[0, :D]
    off = so[0, D:]

    wq2 = wq.astype(f8) * sc[:, None] * SCALE
    cq = (off @ wq.astype(f8)) * SCALE
    wk2 = wk.astype(f8) * sc[:, None]
    ck = off @ wk.astype(f8)
    wv2 = wv.astype(f8) * sc[:, None]
    cv = off @ wv.astype(f8)
    wu2 = w_up.astype(f8) * sc[:, None]
    cu = off @ w_up.astype(f8) + b_up.astype(f8)

    def to32(a):
        return np.ascontiguousarray(a, np.float32)

    def tobf(a):
        return np.ascontiguousarray(a, np.float32).astype(BF)

    dev = {}
    dev["wqT"] = tobf(wq2.reshape(4, 128, H, HD).transpose(1, 0, 2, 3).reshape(128, -1))
    dev["wkT"] = tobf(wk2.reshape(4, 128, H, HD).transpose(1, 0, 2, 3).reshape(128, -1))
    dev["wvN"] = tobf(wv2.reshape(4, 128, D).transpose(1, 0, 2).reshape(128, -1))
    dev["wfN"] = tobf(
        w_final.astype(f8).reshape(H, HD, D).transpose(1, 0, 2).reshape(HD, -1))
    dev["wupT"] = tobf(
        wu2.reshape(4, 128, 16, 128).transpose(1, 0, 2, 3).reshape(128, -1))
    dev["wdnN"] = tobf(
        w_down.astype(f8).reshape(16, 128, D).transpose(1, 0, 2).reshape(128, -1))
    dev["cqB"] = to32(cq.reshape(H, HD).T)
    dev["ckB"] = to32(ck.reshape(H, HD).T)
    dev["cvB"] = to32(np.tile(cv[None, :], (128, 1)))
    dev["cuB"] = to32(cu.reshape(16, 128).T)
    dev["ident"] = np.eye(128, dtype=np.float32).astype(BF)
    dev["epsB"] = np.full((128, 1), EPS, np.float32)

    has_bf = bool(np.any(b_final != 0))
    has_bd = bool(np.any(b_down != 0))
    if has_bf:
        dev["bfB"] = to32(np.tile(b_final[None, :], (128, 1)))
    if has_bd:
        dev["bdB"] = to32(np.tile(b_down[None, :], (128, 1)))

    # global key validity from the diagonal mask (keys of block n)
    kv_global = np.asarray(mask[0, 0, :, 0, 0, :], bool).reshape(NP)

    # per-core x slab + key mask
    per_core = []
    xpad = np.zeros((NP, D), np.float32)
    xpad[:N] = x
    for c in range(NCORES):
        g0 = (STARTS[c] - 1) * BS
        xs = np.zeros((SNODES, D), np.float32)
        km = np.zeros(SNODES, np.float32)
        lo = max(0, -g0)
        hi = min(SNODES, NP - g0)
        xs[lo:hi] = xpad[g0 + lo:g0 + hi]
        kmv = np.zeros(SNODES, bool)
        kmv[lo:hi] = kv_global[g0 + lo:g0 + hi]
        km[:] = kmv.astype(np.float32)
        # sanity: invalid keys only at statically-fixed kt tiles
        km_t = kmv.reshape(KT, 128)
        for kt in range(KT):
            if not km_t[kt].all():
                assert kt in FIXUP_KTS, f"unexpected invalid keys at kt={kt}"
        per_core.append({
            "x_slab": xs,
            "kmask": np.ascontiguousarray(kmv.reshape(KT, 128).T.astype(np.float32)),
        })
    return dev, per_core, has_bf, has_bd


def _run(inputs, trace=False, trace_kwargs=None):
    from concourse.bass_utils import run_bass_kernel_spmd

    import os
    dbg = os.environ.get("KERNEL_DEBUG_STAGE") or None
    rep = int(os.environ.get("KERNEL_REPEAT", "1"))
    dev, per_core, has_bf, has_bd = _prep(inputs)
    key = (has_bf, has_bd, dbg, rep)
    if key not in _PROG_CACHE:
        _PROG_CACHE[key] = _build_program(has_bf, has_bd, debug_stage=dbg,
                                          repeat=rep)
    nc = _PROG_CACHE[key]

    in_maps = []
    for c in range(NCORES):
        m = dict(dev)
        m.update(per_core[c])
        in_maps.append(m)
    kw = {}
    if trace:
        kw["trace"] = True
        if trace_kwargs:
            kw.update(trace_kwargs)
    res = run_bass_kernel_spmd(nc, in_maps, list(range(NCORES)), **kw)

    out = np.zeros((NP, D), np.float32)
    for c in range(NCORES):
        nblk = NB - STARTS[c] if c == NCORES - 1 else STARTS[c + 1] - STARTS[c]
        rows = nblk * BS
        out[STARTS[c] * BS: STARTS[c] * BS + rows] = res.results[c]["out"][:rows]
    x_in = np.asarray(inputs["x"])
    return out[:N].reshape(1, N, D).astype(x_in.dtype), res


def kernel(**inputs):
    out, _ = _run(inputs)
    return out



# revision 49
# speedup vs baseline: 1.3744x; 1.3744x over previous
"""Trainium2 Bass kernel for nn_Block_73615739454081 (tri-block sparse attention + FFN).

Contract: kernel(**inputs) takes FULL unsharded inputs (as produced by
setup_inputs()) and returns the FULL [1, N, D] float32 output.

Strategy (8 NeuronCores, SPMD):
  - Shard the block axis: 161 blocks of 256 tokens. Each core runs an identical
    program over 21 "local" blocks plus a 1-block halo on each side (23-block
    slab). Adjacent cores overlap by one block; the gather keeps a disjoint
    20/20/.../21 split. No collectives: the halo is materialized host-side.
  - Conditioning (scale/offset from global_norm_conditioning) and the attention
    1/sqrt(d) scale are folded into the weight matrices on the host, so the
    device only computes plain LayerNorm stats.
  - Attention is computed in transposed (feature-major) layouts throughout:
    qT/kT = [HD, tokens], scores ST = [keys, q], so no transposes are needed
    inside the attention core. Softmax skips max-subtraction (logits are O(1));
    denominators come from ones-matmuls, broadcast back via a K=1 matmul.
  - Key-validity masking (padding tokens / missing neighbor blocks) is data
    driven: exp(scores) rows are multiplied by a per-core 0/1 key-mask at the
    (statically known) slab edges.
  - QKV/final/FFN matmuls run in fp8e4m3 DoubleRow mode (K=256 per pass,
    ~1.8x the bf16 rate at FD=512); weights are pre-scaled by WS=64 to
    escape fp8 subnormals, compensated via the exp() input scale (q,k),
    a ones=WS softmax-denominator trick (v) and the gelu input scale
    (w_up).  Scores/AV stay bf16 (K=128: DoubleRow is LDWEIGHTS-bound
    there).  Accumulation is fp32 in PSUM; the residual path stays fp32.
  - LayerNorm rstd is computed entirely on the vector engine (bit-hack
    rsqrt seed + one Newton step, batched across subtiles) so the scalar
    engine's activation table never leaves the Exp/Gelu sets -- each
    table switch costs ~1.3us.
  - Emission is software-pipelined at the chain level: the next blocks'
    LN runs under the current chain's scores/AV; both blocks' transposes
    (and PSUM->SBUF copies) are emitted before rest_A so they precede the
    LN2 chain in the engine FIFOs; QKV projections after rest_A cover the
    LN2 latency; the FFN follows.  This keeps the tensor queue fed across
    every serial DVE/ACT chain so the PE never idles long enough for the
    HAM clock gate to re-throttle it to half rate.
"""

import numpy as np

# ---------------------------------------------------------------- constants
N = 40962
D = 512
H = 4
HD = 128
BS = 256
NB = 161
NP = NB * BS
C = 16
FFW = 4 * D
SCALE = HD ** -0.5
EPS = 1e-5
WS = 64.0        # fp8 weight pre-scale; folded back out on device (exp
                 # scale for q/k, ones=WS denominator trick for v, gelu
                 # input scale for the FFN-up path)

NCORES = 8
LOCAL = 21                 # local blocks per core (uniform SPMD program)
SLAB = LOCAL + 2           # +1 halo block each side
SNODES = SLAB * BS         # 5888 slab tokens
KT = SNODES // 128         # 46 key tiles of 128 in the slab
OUT_NODES = LOCAL * BS     # 5376
STARTS = [0, 20, 40, 60, 80, 100, 120, 140]   # first local block per core
# kt tiles that can contain invalid keys on some core (slab edges):
FIXUP_KTS = (0, 1, KT - 4, KT - 3, KT - 2, KT - 1)

_PROG_CACHE = {}


# ---------------------------------------------------------------- device code
def _build_program(has_bf, has_bd, debug_stage=None, repeat=1):
    import concourse.bass as bass  # noqa: F401
    import concourse.mybir as mybir
    import concourse.tile as tile
    from concourse import bacc

    F32 = mybir.dt.float32
    BF16 = mybir.dt.bfloat16
    AF = mybir.ActivationFunctionType
    OP = mybir.AluOpType

    nc = bacc.Bacc("TRN2", target_bir_lowering=False, debug=False)

    def din(name, shape, dt):
        return nc.dram_tensor(name, shape, dt, kind="ExternalInput").ap()

    F8 = mybir.dt.float8e4
    DR = mybir.MatmulPerfMode.DoubleRow
    x_d = din("x_slab", [SNODES, D], F32)
    km_d = din("kmask", [128, KT], F32)
    wqT_d = din("wqT", [128, 4, H * HD], F8)
    wkT_d = din("wkT", [128, 4, H * HD], F8)
    wvN_d = din("wvN", [128, 4, D], F8)
    wfN_d = din("wfN", [128, 4, D], F8)
    wupT_d = din("wupT", [128, 4, FFW], F8)
    wdnN_d = din("wdnN", [128, 16, D], F8)
    cqB_d = din("cqB", [128, H], F32)
    ckB_d = din("ckB", [128, H], F32)
    cvB_d = din("cvB", [128, D], F32)
    cuB_d = din("cuB", [128, 16], F32)
    ident_d = din("ident", [128, 128], BF16)
    epsB_d = din("epsB", [128, 1], F32)
    if has_bf:
        bfB_d = din("bfB", [128, D], F32)
    if has_bd:
        bdB_d = din("bdB", [128, D], F32)
    out_d = nc.dram_tensor("out", [OUT_NODES, D], F32, kind="ExternalOutput").ap()

    with nc.allow_low_precision(reason="bf16 matmul operands by design"), \
         tile.TileContext(nc) as tc:
        with (
            tc.tile_pool(name="wconst", bufs=1) as wpool,
            tc.tile_pool(name="sb", bufs=2) as sb,
            tc.tile_pool(name="ps", bufs=2, space="PSUM") as ps,
        ):
            # ---------------- resident weights / constants
            # small constants first, then weights in first-use order so the
            # first phase1 blocks only on a small DMA prefix.
            epsB = wpool.tile([128, 1], F32, name="epsB")
            nc.scalar.dma_start(out=epsB, in_=epsB_d)
            ident = wpool.tile([128, 128], BF16, name="ident")
            nc.scalar.dma_start(out=ident, in_=ident_d)
            cqB = wpool.tile([128, H], F32, name="cqB")
            nc.scalar.dma_start(out=cqB, in_=cqB_d)
            ckB = wpool.tile([128, H], F32, name="ckB")
            nc.scalar.dma_start(out=ckB, in_=ckB_d)
            cvB = wpool.tile([128, D], F32, name="cvB")
            nc.scalar.dma_start(out=cvB, in_=cvB_d)
            cuB = wpool.tile([128, 16], F32, name="cuB")
            nc.scalar.dma_start(out=cuB, in_=cuB_d)
            kmt = wpool.tile([128, KT], F32, name="kmt")
            nc.scalar.dma_start(out=kmt, in_=km_d)
            if has_bf:
                bfB = wpool.tile([128, D], F32, name="bfB")
                nc.scalar.dma_start(out=bfB, in_=bfB_d)
            if has_bd:
                bdB = wpool.tile([128, D], F32, name="bdB")
                nc.scalar.dma_start(out=bdB, in_=bdB_d)
            wkT = wpool.tile([128, 4, H * HD], F8, name="wkT")
            nc.scalar.dma_start(out=wkT, in_=wkT_d)
            wvN = wpool.tile([128, 4, D], F8, name="wvN")
            nc.scalar.dma_start(out=wvN, in_=wvN_d)
            wqT = wpool.tile([128, 4, H * HD], F8, name="wqT")
            nc.scalar.dma_start(out=wqT, in_=wqT_d)
            wfN = wpool.tile([128, 4, D], F8, name="wfN")
            nc.scalar.dma_start(out=wfN, in_=wfN_d)
            wupT = wpool.tile([128, 4, FFW], F8, name="wupT")
            nc.scalar.dma_start(out=wupT, in_=wupT_d)
            wdnN = wpool.tile([128, 16, D], F8, name="wdnN")
            nc.scalar.dma_start(out=wdnN, in_=wdnN_d)

            kT_ring = {}
            v_ring = {}
            qT_ring = {}

            def ln_group(srcs, nm, apply_on_act=False):
                """Batched LN for several [128,512] subtiles: stats per
                subtile, then ONE rsqrt Newton chain on a packed [128,n]
                tile (DVE-only bit-hack seed: keeping Sqrt off ACT means
                the only activation-table switches left are Exp<->Gelu).
                """
                n = len(srcs)
                mvs = []
                for i, src in enumerate(srcs):
                    stats = sb.tile([128, 6], F32, tag="stats", bufs=8,
                                    name=f"st{nm}_{i}")
                    nc.vector.bn_stats(out=stats, in_=src)
                    mv = sb.tile([128, 2], F32, tag="mv", bufs=8,
                                 name=f"mv{nm}_{i}")
                    nc.vector.bn_aggr(out=mv, in_=stats)
                    mvs.append(mv)
                ve = sb.tile([128, n], F32, tag="std", bufs=8, name=f"sd{nm}")
                for i, mv in enumerate(mvs):
                    nc.vector.tensor_scalar(ve[:, i:i + 1], mv[:, 1:2], EPS,
                                            None, OP.add)
                rstd = sb.tile([128, n], F32, tag="rstd", bufs=8, name=f"rs{nm}")
                t = sb.tile([128, n], F32, tag="rsA", bufs=8, name=f"ra{nm}")
                h = sb.tile([128, n], F32, tag="rsB", bufs=8, name=f"rb{nm}")
                U32 = mybir.dt.uint32
                I32 = mybir.dt.int32
                nc.vector.tensor_scalar(t.bitcast(U32), ve.bitcast(U32),
                                        1, None, OP.logical_shift_right)
                # seed = bitcast(M - (bits>>1)); the subtract runs in fp32
                # VALUE space (DVE int arithmetic saturates via fp32, so a
                # u32 wrap-around add is not available).
                nc.vector.tensor_copy(h, t.bitcast(I32))
                nc.vector.tensor_scalar(h, h, float(0x5F375A86), -1.0,
                                        OP.subtract, OP.mult)
                nc.vector.tensor_copy(rstd.bitcast(I32), h)
                nc.vector.tensor_scalar(h, ve, -0.5, None, OP.mult)
                nc.vector.tensor_tensor(t, rstd, rstd, OP.mult)
                nc.vector.tensor_tensor(t, t, h, OP.mult)
                nc.vector.tensor_scalar(t, t, 1.5, None, OP.add)
                nc.vector.tensor_tensor(rstd, rstd, t, OP.mult)
                hns = []
                for i, src in enumerate(srcs):
                    hn = sb.tile([128, D], BF16, tag="hn", bufs=18,
                                 name=f"hn{nm}_{i}")
                    rs = rstd[:, i:i + 1]
                    if apply_on_act:
                        nmr = sb.tile([128, 1], F32, tag="nmr", bufs=8,
                                      name=f"nm{nm}_{i}")
                        nc.vector.tensor_scalar(
                            nmr, mvs[i][:, 0:1], rs, -1.0, OP.mult, OP.mult)
                        nc.scalar.activation(hn, src, AF.Identity, bias=nmr,
                                             scale=rs)
                    else:
                        nc.vector.tensor_scalar(
                            hn, src, mvs[i][:, 0:1], rs, OP.subtract, OP.mult)
                    hns.append(hn)
                return hns

            xt_ring = {}
            hn_ring = {}

            def fetch(j):
                """Prefetch x rows for slab block j into the xt ring."""
                r0 = j * BS
                for nt in range(2):
                    xt = sb.tile([128, D], F32, tag="xin", bufs=12, name=f"x{j}_{nt}")
                    nc.sync.dma_start(out=xt, in_=x_d[r0 + nt * 128:r0 + nt * 128 + 128, :])
                    xt_ring[(j, nt)] = xt

            def ln_phase(j):
                """LN1 for block j (DVE/ACT only) -> hn ring."""
                hn_ring[j] = ln_group([xt_ring[(j, nt)] for nt in range(2)],
                                      f"1_{j}")

            # v_aug layout: per-head 129-wide slot [ones | v_h] so the AV
            # matmul's rhs yields the softmax denominator in col 0.  The 10
            # slots are persistent and memset ONCE here (the adds only ever
            # write cols 1..128 of each slot), keeping the slow GPSIMD memset
            # out of every block's WAR chain.
            v_slots = []
            for si in range(10):
                vs = wpool.tile([128, H * (HD + 1)], BF16, name=f"vslot{si}")
                nc.gpsimd.memset(vs, WS)
                v_slots.append(vs)

            h1_ring = {}

            def mm_T(j):
                """Transposes for block j into its PAIR's h1T tile
                [128, 4(dt), 512] fp8 -- block pairs share one tile so the
                q/k projections can stream 512 tokens per DoubleRow matmul."""
                hn = hn_ring.pop(j)
                jp = j - (j % 2)
                if jp not in h1_ring:
                    h1_ring[jp] = sb.tile([128, 4, 2 * BS], F8, tag="h1T",
                                          bufs=2, name=f"h1T{jp}")
                h1T = h1_ring[jp]
                boff = (j % 2) * BS
                for g in range(2):  # two psum tiles, each covers 2 d-tiles
                    pT = ps.tile([128, 512], BF16,
                                 tag=("p1" if g else "st"), bufs=2,
                                 name=f"pT{j}_{g}")
                    for dl in range(2):
                        dt = g * 2 + dl
                        for nt in range(2):
                            nc.tensor.transpose(
                                pT[:, dl * 256 + nt * 128:dl * 256 + nt * 128 + 128],
                                hn[nt][:, dt * 128:dt * 128 + 128],
                                ident,
                            )
                    for dl in range(2):
                        dt = g * 2 + dl
                        dst = h1T[:, dt:dt + 1, boff:boff + BS].squeeze(1)
                        src = pT[:, dl * 256:dl * 256 + 256]
                        if dl == 0:
                            nc.vector.tensor_copy(dst, src)
                        else:
                            nc.scalar.copy(dst, src)

            def proj_pair(jp, nblk):
                """q/k DoubleRow fp8 projections for blocks jp..jp+nblk-1."""
                W = nblk * BS
                h1T = h1_ring.pop(jp)
                outs = {}
                for bi in range(nblk):
                    if 1 <= jp + bi <= LOCAL:
                        outs[("q", bi)] = sb.tile(
                            [128, H * BS], BF16, tag="qring", bufs=3,
                            name=f"qT{jp + bi}")
                    outs[("k", bi)] = sb.tile(
                        [128, H * BS], BF16, tag="kring", bufs=4,
                        name=f"kT{jp + bi}")
                do_q = any(("q", bi) in outs for bi in range(nblk))
                for h in range(H):
                    # q/k interleaved per head: doubles the p1 reuse distance
                    # so a fresh psum never waits on the bias-add reads of
                    # the one right before it.
                    for wT3, nm in ((wqT, "q"), (wkT, "k")):
                        if nm == "q" and not do_q:
                            continue
                        # alternate p1/st (st is idle between rest_A and
                        # rest_B): 4 rotating psum banks, so the PE never
                        # waits on the bias-add drain of the previous group
                        # when the DVE is busy with the LN stats burst.
                        pQ = ps.tile([128, W], F32,
                                     tag=("p1" if (2 * h + (nm == "k")) % 2
                                          else "st"), bufs=2,
                                     name=f"p{nm}{jp}_{h}")
                        for kp in range(2):
                            nc.tensor.matmul(
                                pQ,
                                lhsT=wT3[:, 2 * kp:2 * kp + 2, h * HD:h * HD + HD],
                                rhs=h1T[:, 2 * kp:2 * kp + 2, 0:W],
                                start=(kp == 0), stop=(kp == 1), perf_mode=DR)
                        cB = cqB if nm == "q" else ckB
                        for bi in range(nblk):
                            if (nm, bi) not in outs:
                                continue
                            dst = outs[(nm, bi)][:, h * 256:h * 256 + 256]
                            src = pQ[:, bi * 256:bi * 256 + 256]
                            if (h + bi) % 2 == 0:
                                nc.vector.tensor_scalar(
                                    dst, src, cB[:, h:h + 1], None, OP.add)
                            else:
                                nc.scalar.add(dst, src, cB[:, h:h + 1])
                for bi in range(nblk):
                    if ("q", bi) in outs:
                        qT_ring[jp + bi] = outs[("q", bi)]
                    kT_ring[jp + bi] = outs[("k", bi)]
                return h1T

            def v_pair(jp, nblk, h1T):
                """V DoubleRow fp8 projection for blocks jp..jp+nblk-1."""
                for bi in range(nblk):
                    for nt in range(2):
                        j = jp + bi
                        # continue the 3-pool rotation started by the
                        # q/k projections so pV never reuses a slot whose
                        # bias-add reader is still queued behind LN2.
                        pV = ps.tile([128, D], F32,
                                     tag=("st", "p1", "ac")[
                                         (2 + 2 * bi + nt) % 3], bufs=2,
                                     name=f"pV{j}_{nt}")
                        for kp in range(2):
                            nc.tensor.matmul(
                                pV,
                                lhsT=h1T[:, 2 * kp:2 * kp + 2,
                                         bi * BS + nt * 128:bi * BS + nt * 128 + 128],
                                rhs=wvN[:, 2 * kp:2 * kp + 2, :],
                                start=(kp == 0), stop=(kp == 1), perf_mode=DR)
                        vt = v_slots[(2 * j + nt) % 10]
                        for h in range(H):
                            nc.vector.tensor_tensor(
                                vt[:, h * 129 + 1:h * 129 + 129],
                                pV[:, h * 128:h * 128 + 128],
                                cvB[:, h * 128:h * 128 + 128], OP.add)
                        v_ring[(j, nt)] = vt

            def chain_attn(b0, nb):
                """Attention for local blocks b0..b0+nb-1.

                Scores stay [keys, q]; the AV matmul uses exp(S) as the
                STATIONARY operand against the augmented V ([ones | v_h]
                slots), producing o = [q, 1 + hd] per query subtile with the
                softmax denominator in column 0.  Normalization is then a
                [128,1] reciprocal + per-partition scale (DVE-only, so the
                tensor engine never waits on it), and a PE transpose brings
                o back to [hd, q] for the final projection.  Returns
                (oTn, finisher); the caller emits finisher() later so the
                last head's transposes hide behind independent matmuls.
                """
                qw = nb * BS
                nqs = 2 * nb
                kts = list(range(2 * b0, 2 * b0 + 2 * (nb + 2)))

                def _vbis(kt):
                    return [bi for bi in range(nb)
                            if 2 * (b0 + bi) <= kt <= 2 * (b0 + bi) + 5]

                # prefetch the residual rows rest_A will need, so the adds
                # there never wait on DMA latency
                xres_t = []
                for qs in range(nqs):
                    xres = sb.tile([128, D], F32, tag="xres", bufs=8,
                                   name=f"xr{b0}_{qs}")
                    r0 = (b0 + 1) * BS + qs * 128
                    nc.sync.dma_start(out=xres, in_=x_d[r0:r0 + 128, :])
                    xres_t.append(xres)

                # head-PAIR fp8 oT tiles [128, 2(head), qw] so the final
                # projection can run DoubleRow over head pairs.
                oTp = [sb.tile([128, 2, qw], F8, tag="oTn", bufs=4,
                               name=f"oT{b0}_{hp}") for hp in range(2)]
                pend = None
                for h in range(H):
                    # ---- scores + exp for every kt of this head
                    Et = {}
                    for ki, kt in enumerate(kts):
                        vbis = _vbis(kt)
                        c0 = min(vbis) * BS
                        c1 = (max(vbis) + 1) * BS
                        # alternate between the "st" and "p1" pools (p1 is
                        # idle during the chain): 4 rotating score banks, so
                        # a score matmul never waits on the exp read of the
                        # tile issued right before it.
                        STp = ps.tile([128, qw], F32,
                                      tag=("st" if ki % 2 else "p1"), bufs=2,
                                      name=f"S{b0}_{h}_{kt}")
                        ksrc = kT_ring[kt // 2][:, h * 256 + (kt % 2) * 128:
                                                h * 256 + (kt % 2) * 128 + 128]
                        for bi in vbis:
                            nc.tensor.matmul(
                                STp[:, bi * BS:bi * BS + BS],
                                lhsT=ksrc,
                                rhs=qT_ring[b0 + bi + 1][:, h * 256:h * 256 + 256],
                                start=True, stop=True,
                            )
                        E = sb.tile([128, qw], BF16, tag="E", bufs=10,
                                    name=f"E{b0}_{h}_{kt}")
                        nc.scalar.activation(E[:, c0:c1], STp[:, c0:c1], AF.Exp,
                                             scale=1.0 / (WS * WS))
                        if kt in FIXUP_KTS:
                            nc.vector.tensor_scalar(
                                E[:, c0:c1], E[:, c0:c1],
                                kmt[:, kt:kt + 1], None, OP.mult
                            )
                        Et[kt] = E
                    if pend is not None:
                        pend()      # prev head's transposes, covered by scores
                        pend = None
                    # ---- AV': per query subtile, accumulate over its 6 kts
                    oah = [ps.tile([128, 2 * (HD + 1)], F32, tag="oacc", bufs=2,
                                   name=f"oa{b0}_{h}_{hf}") for hf in range(nb)]
                    for qs in range(nqs):
                        bi = qs // 2
                        myk = list(range(2 * (b0 + bi), 2 * (b0 + bi) + 6))
                        dst = oah[qs // 2][:, (qs % 2) * 129:(qs % 2) * 129 + 129]
                        for i, kt in enumerate(myk):
                            nc.tensor.matmul(
                                dst,
                                lhsT=Et[kt][:, qs * 128:qs * 128 + 128],
                                rhs=v_ring[(kt // 2, kt % 2)][:, h * 129:h * 129 + 129],
                                start=(i == 0), stop=(i == 5),
                            )
                    # ---- normalize (reciprocal on DVE; the scale split
                    # DVE/ACT to balance engine load)
                    oos = []
                    for qs in range(nqs):
                        src = oah[qs // 2][:, (qs % 2) * 129:(qs % 2) * 129 + 129]
                        rdn = sb.tile([128, 1], F32, tag="rdn", bufs=8,
                                      name=f"rd{b0}_{h}_{qs}")
                        nc.vector.reciprocal(rdn, src[:, 0:1])
                        oo = sb.tile([128, HD], BF16, tag="oon", bufs=8,
                                     name=f"oo{b0}_{h}_{qs}")
                        nc.vector.tensor_scalar(oo, src[:, 1:129], rdn,
                                                None, OP.mult)
                        oos.append(oo)

                    def mk(h=h, oos=oos, nqs=nqs, qw=qw, b0=b0):
                        trp = ps.tile([128, qw], BF16, tag="oacc", bufs=2,
                                      name=f"tr{b0}_{h}")
                        for qs in range(nqs):
                            nc.tensor.transpose(
                                trp[:, qs * 128:qs * 128 + 128], oos[qs], ident)
                        dst = oTp[h // 2][:, h % 2:h % 2 + 1, :].squeeze(1)
                        if h % 2 == 0:
                            nc.vector.tensor_copy(dst, trp)
                        else:
                            nc.scalar.copy(dst, trp)
                    pend = mk
                return (oTp, xres_t), pend

            def rest_A(b0, nb, oTp, xres_t):
                """Final projection (fp8 DoubleRow over head pairs) +
                residual + LN2 for local blocks."""
                qw = nb * BS
                nqs = qw // 128
                r1 = []
                hn2 = []
                for qs in range(nqs):
                    at = ps.tile([128, D], F32, tag="ac", bufs=2, name=f"at{b0}_{qs}")
                    for hp in range(2):
                        nc.tensor.matmul(
                            at,
                            lhsT=oTp[hp][:, 0:2, qs * 128:qs * 128 + 128],
                            rhs=wfN[:, 2 * hp:2 * hp + 2, :],
                            start=(hp == 0), stop=(hp == 1), perf_mode=DR,
                        )
                    rt = sb.tile([128, D], F32, tag="r1", bufs=8, name=f"r1{b0}_{qs}")
                    nc.vector.tensor_tensor(rt, at, xres_t[qs], OP.add)
                    if has_bf:
                        nc.vector.tensor_tensor(rt, rt, bfB, OP.add)
                    r1.append(rt)
                    if debug_stage == "r1":
                        ro = b0 * BS + qs * 128
                        nc.sync.dma_start(out=out_d[ro:ro + 128, :], in_=rt)
                if debug_stage != "r1":
                    hn2 = ln_group(r1, f"2_{b0}", apply_on_act=True)
                return (b0, nb, r1, hn2)

            def rest_B(state):
                """h2T transposes + FFN; emitted after the NEXT chain's
                attention so the LN2 serial chain hides behind matmuls."""
                b0, nb, r1, hn2 = state
                if debug_stage == "r1":
                    return
                qw = nb * BS
                nqs = qw // 128
                h2T3 = sb.tile([128, 4, qw], F8, tag="h2T", bufs=3,
                               name=f"h2{b0}")
                for dt in range(4):
                    hps = ps.tile([128, qw], BF16, tag="ac", bufs=2,
                                  name=f"hp{b0}_{dt}")
                    for qs in range(nqs):
                        nc.tensor.transpose(
                            hps[:, qs * 128:qs * 128 + 128],
                            hn2[qs][:, dt * 128:dt * 128 + 128],
                            ident,
                        )
                    dst = h2T3[:, dt:dt + 1, :].squeeze(1)
                    if dt % 2 == 0:
                        nc.vector.tensor_copy(dst, hps)
                    else:
                        nc.scalar.copy(dst, hps)

                # FFN up (DoubleRow over d-tile pairs) + gelu into fb-pair
                # fp8 tiles, then down (DoubleRow over fb pairs) per subtile
                glp = [sb.tile([128, 2, qw], F8, tag="gl", bufs=9,
                               name=f"gl{b0}_{p}") for p in range(8)]
                for fb in range(16):
                    g = ps.tile([128, qw], F32, tag=("ac" if fb % 2 else "st"),
                                bufs=2, name=f"g{b0}_{fb}")
                    for kp in range(2):
                        nc.tensor.matmul(
                            g,
                            lhsT=wupT[:, 2 * kp:2 * kp + 2, fb * 128:fb * 128 + 128],
                            rhs=h2T3[:, 2 * kp:2 * kp + 2, :],
                            start=(kp == 0), stop=(kp == 1), perf_mode=DR,
                        )
                    gt = glp[fb // 2][:, fb % 2:fb % 2 + 1, :].squeeze(1)
                    nc.scalar.activation(gt, g, AF.Gelu_apprx_tanh,
                                         bias=cuB[:, fb:fb + 1], scale=1.0 / WS)
                for qs in range(nqs):
                    y = ps.tile([128, D], F32, tag="ac", bufs=2, name=f"y{b0}_{qs}")
                    for p in range(8):
                        nc.tensor.matmul(
                            y,
                            lhsT=glp[p][:, 0:2, qs * 128:qs * 128 + 128],
                            rhs=wdnN[:, 2 * p:2 * p + 2, :],
                            start=(p == 0), stop=(p == 7), perf_mode=DR,
                        )
                    ot = sb.tile([128, D], F32, tag="outt", bufs=3,
                                 name=f"ot{b0}_{qs}")
                    nc.vector.tensor_tensor(ot, y, r1[qs], OP.add)
                    if has_bd:
                        nc.vector.tensor_tensor(ot, ot, bdB, OP.add)
                    ro = b0 * BS + qs * 128
                    nc.sync.dma_start(out=out_d[ro:ro + 128, :], in_=ot)

            # ---------------- emission (software-pipelined at chain level)
            # Cycle c:
            #   fetch(2c+6..7)  ln(2c+4..5)   <- LN1 runs on DVE/ACT under
            #                                    chain(c)'s PE burst
            #   chain_attn(c)   fin(c)
            #   mm_qk(2c+4)                   <- hides fin's oT copy latency
            #   rest_A(c)                     <- final proj; LN2 chain starts
            #   mm_v(2c+4) mm_qk/v(2c+5)      <- ~7us of matmuls covering the
            #                                    LN2 serial chain
            #   rest_B(c)                     <- h2T + FFN (LN2 ready)
            # This keeps the tensor queue fed across every serial DVE/ACT
            # chain, so the PE never idles long enough for HAM to rethrottle.
            for _rep in range(repeat):
                kT_ring.clear(); v_ring.clear(); qT_ring.clear()
                xt_ring.clear(); hn_ring.clear(); h1_ring.clear()
                for j in range(6):
                    fetch(j)
                for j in range(6):
                    ln_phase(j)
                for jp in (0, 2):
                    mm_T(jp)
                    mm_T(jp + 1)
                    v_pair(jp, 2, proj_pair(jp, 2))
                for c in range(10):
                    # LN1 a full cycle ahead of its mm_T consumer: the
                    # bn_stats burst drains on the DVE during the chain's
                    # light stretch instead of blocking the projection
                    # phase's dependency chain.
                    for j in (2 * c + 6, 2 * c + 7):
                        if j < SLAB:
                            fetch(j)
                            ln_phase(j)
                    (oTp, xres_t), fin = chain_attn(2 * c, 2)
                    fin()
                    # both blocks' transposes (and their PSUM->SBUF copies)
                    # BEFORE rest_A: the copies then precede the LN2 chain in
                    # the DVE/ACT FIFOs, so the q/k/v projections right after
                    # rest_A find h1T ready instead of stalling ~4us.
                    for j in (2 * c + 4, 2 * c + 5):
                        if j < SLAB:
                            mm_T(j)
                    st = rest_A(2 * c, 2, oTp, xres_t)
                    if 2 * c + 5 < SLAB:
                        v_pair(2 * c + 4, 2, proj_pair(2 * c + 4, 2))
                    elif 2 * c + 4 < SLAB:
                        # odd tail block (the far halo): single-block pair
                        v_pair(2 * c + 4, 1, proj_pair(2 * c + 4, 1))
                    if c < 9:
                        rest_B(st)
                    else:
                        # tail: emit the last (nb=1) chain before rest_B(9)
                        # so its LN2 serial chain is covered by the FFN of
                        # chain 9 instead of stalling the tensor queue.
                        (oTp1, xres1), fin1 = chain_attn(20, 1)
                        fin1()
                        st1 = rest_A(20, 1, oTp1, xres1)
                        rest_B(st)
                        rest_B(st1)

    nc.compile()
    return nc


# ---------------------------------------------------------------- host side
def _prep(inputs):
    import ml_dtypes
    f8 = np.float64
    BF = ml_dtypes.bfloat16
    x = np.asarray(inputs["x"], np.float32).reshape(N, D)
    gnc = np.asarray(inputs["global_norm_conditioning"], np.float32)
    mask = np.asarray(inputs["mask"])
    wq = np.asarray(inputs["wq"], np.float32)
    wk = np.asarray(inputs["wk"], np.float32)
    wv = np.asarray(inputs["wv"], np.float32)
    w_final = np.asarray(inputs["w_final"], np.float32)
    b_final = np.asarray(inputs["b_final"], np.float32)
    w_up = np.asarray(inputs["w_up"], np.float32)
    b_up = np.asarray(inputs["b_up"], np.float32)
    w_down = np.asarray(inputs["w_down"], np.float32)
    b_down = np.asarray(inputs["b_down"], np.float32)
    w_cond = np.asarray(inputs["w_cond"], np.float32)
    b_cond = np.asarray(inputs["b_cond"], np.float32)

    so = gnc.astype(f8) @ w_cond.astype(f8) + b_cond.astype(f8)
    sc = 1.0 + so[0, :D]
    off = so[0, D:]

    # fp8 weight pre-scale WS: wq/wk compensated in the exp() input scale
    # (1/WS^2), wv via the ones=WS denominator trick, w_up via the gelu
    # input scale (1/WS).  w_final / w_down stay unscaled fp8 (their ~0.02
    # sigma quantizes to ~3% rel err; contributions are small vs the gate).
    wq2 = wq.astype(f8) * sc[:, None] * SCALE * WS
    cq = (off @ wq.astype(f8)) * SCALE * WS
    wk2 = wk.astype(f8) * sc[:, None] * WS
    ck = (off @ wk.astype(f8)) * WS
    wv2 = wv.astype(f8) * sc[:, None] * WS
    cv = (off @ wv.astype(f8)) * WS
    wu2 = w_up.astype(f8) * sc[:, None] * WS
    cu = off @ w_up.astype(f8) + b_up.astype(f8)
    wf2 = w_final.astype(f8)
    wd2 = w_down.astype(f8)

    def to32(a):
        return np.ascontiguousarray(a, np.float32)

    def tobf(a):
        return np.ascontiguousarray(a, np.float32).astype(BF)

    F8NP = ml_dtypes.float8_e4m3

    def tof8(a):
        return np.ascontiguousarray(
            np.clip(a, -240, 240), np.float32).astype(F8NP)

    dev = {}
    dev["wqT"] = tof8(wq2.reshape(4, 128, H, HD).transpose(1, 0, 2, 3).reshape(128, 4, H * HD))
    dev["wkT"] = tof8(wk2.reshape(4, 128, H, HD).transpose(1, 0, 2, 3).reshape(128, 4, H * HD))
    dev["wvN"] = tof8(wv2.reshape(4, 128, D).transpose(1, 0, 2))
    dev["wfN"] = tof8(wf2.reshape(H, HD, D).transpose(1, 0, 2))
    dev["wupT"] = tof8(
        wu2.reshape(4, 128, 16, 128).transpose(1, 0, 2, 3).reshape(128, 4, FFW))
    dev["wdnN"] = tof8(wd2.reshape(16, 128, D).transpose(1, 0, 2))
    dev["cqB"] = to32(cq.reshape(H, HD).T)
    dev["ckB"] = to32(ck.reshape(H, HD).T)
    dev["cvB"] = to32(np.tile(cv[None, :], (128, 1)))
    dev["cuB"] = to32(cu.reshape(16, 128).T)
    dev["ident"] = np.eye(128, dtype=np.float32).astype(BF)
    dev["epsB"] = np.full((128, 1), EPS, np.float32)

    has_bf = bool(np.any(b_final != 0))
    has_bd = bool(np.any(b_down != 0))
    if has_bf:
        dev["bfB"] = to32(np.tile(b_final[None, :], (128, 1)))
    if has_bd:
        dev["bdB"] = to32(np.tile(b_down[None, :], (128, 1)))

    # global key validity from the diagonal mask (keys of block n)
    kv_global = np.asarray(mask[0, 0, :, 0, 0, :], bool).reshape(NP)

    # per-core x slab + key mask
    per_core = []
    xpad = np.zeros((NP, D), np.float32)
    xpad[:N] = x
    for c in range(NCORES):
        g0 = (STARTS[c] - 1) * BS
        xs = np.zeros((SNODES, D), np.float32)
        km = np.zeros(SNODES, np.float32)
        lo = max(0, -g0)
        hi = min(SNODES, NP - g0)
        xs[lo:hi] = xpad[g0 + lo:g0 + hi]
        kmv = np.zeros(SNODES, bool)
        kmv[lo:hi] = kv_global[g0 + lo:g0 + hi]
        km[:] = kmv.astype(np.float32)
        # sanity: invalid keys only at statically-fixed kt tiles
        km_t = kmv.reshape(KT, 128)
        for kt in range(KT):
            if not km_t[kt].all():
                assert kt in FIXUP_KTS, f"unexpected invalid keys at kt={kt}"
        per_core.append({
            "x_slab": xs,
            "kmask": np.ascontiguousarray(kmv.reshape(KT, 128).T.astype(np.float32)),
        })
    return dev, per_core, has_bf, has_bd


def _run(inputs, trace=False, trace_kwargs=None):
    from concourse.bass_utils import run_bass_kernel_spmd

    import os
    dbg = os.environ.get("KERNEL_DEBUG_STAGE") or None
    rep = int(os.environ.get("KERNEL_REPEAT", "1"))
    dev, per_core, has_bf, has_bd = _prep(inputs)
    key = (has_bf, has_bd, dbg, rep)
    if key not in _PROG_CACHE:
        _PROG_CACHE[key] = _build_program(has_bf, has_bd, debug_stage=dbg,
                                          repeat=rep)
    nc = _PROG_CACHE[key]

    in_maps = []
    for c in range(NCORES):
        m = dict(dev)
        m.update(per_core[c])
        in_maps.append(m)
    kw = {}
    if trace:
        kw["trace"] = True
        if trace_kwargs:
            kw.update(trace_kwargs)
    res = run_bass_kernel_spmd(nc, in_maps, list(range(NCORES)), **kw)

    out = np.zeros((NP, D), np.float32)
    for c in range(NCORES):
        nblk = NB - STARTS[c] if c == NCORES - 1 else STARTS[c + 1] - STARTS[c]
        rows = nblk * BS
        out[STARTS[c] * BS: STARTS[c] * BS + rows] = res.results[c]["out"][:rows]
    x_in = np.asarray(inputs["x"])
    return out[:N].reshape(1, N, D).astype(x_in.dtype), res


def kernel(**inputs):
    out, _ = _run(inputs)
    return out

